# revision 4
# baseline (speedup 1.0000x reference)
"""LongRangeProj Bass kernel for TRN2 (8 NeuronCores, channel-sharded).

Math: out[b,c,h,w] = max_{o=(i,j)} x[b,c,o] * exp(-(inv2rv*(fn-|rm|)^2
                                                   + inv2av*wrap(theta-a)^2))
with fn/theta = polar coords of pixel (h,w) around origin o, and the angle
term forced to 1 at the origin pixel itself (mask).  exp is monotone, so the
max is taken on the exponent and exp applied to the reduced [B,C,H,W] only.

Per-core layout: partitions = 2 batches x 64 origins, free dim = 4096 pixels.
Each core owns C/8 = 8 channels; one channel per iteration.
Engines: ACT (affine+Square+Exp, one table set), DVE (sub/add + PSUM max
reduce), GPSIMD (round-trick + mask mul), PE (128x128 fp32 transposes).
"""

import numpy as np
from contextlib import ExitStack

B, C, NH, NW, H, W = 2, 64, 8, 8, 64, 64
STRIDE = 8
NCORES = 8
CL = C // NCORES          # channels per core
HW = H * W                # 4096
NO = NH * NW              # 64 origins
FREE_CHUNK = 2048
NBLK = HW // 128          # 32 pixel blocks of 128
CBIG = float(1.5 * 2 ** 23)   # fp32 round-to-nearest magic constant
TWO_PI = 2.0 * np.pi

_built = {}


def _host_fields():
    """Constant geometric fields in [NO, HW] layout, fp32."""
    oy = np.arange(NH, dtype=np.float64) * STRIDE
    ox = np.arange(NW, dtype=np.float64) * STRIDE
    yg = np.arange(H, dtype=np.float64)
    xg = np.arange(W, dtype=np.float64)
    fy = yg[None, :] - oy[:, None]                      # [NH, H]
    fx = xg[None, :] - ox[:, None]                      # [NW, W]
    FY = np.broadcast_to(fy[:, None, :, None], (NH, NW, H, W))
    FX = np.broadcast_to(fx[None, :, None, :], (NH, NW, H, W))
    fn = np.sqrt(FX * FX + FY * FY)
    theta = np.arctan2(FY, FX)
    v = theta / TWO_PI
    mask = np.zeros((NH, NW, H, W), dtype=np.float64)
    for i in range(NH):
        for j in range(NW):
            mask[i, j, i * STRIDE, j * STRIDE] = 1.0
    notm = 1.0 - mask
    rs = lambda a: np.ascontiguousarray(a.reshape(NO, HW).astype(np.float32))
    return rs(v), rs(fn), rs(notm)


def _build_bass():
    import concourse.bass as bass
    import concourse.bacc as bacc
    import concourse.tile as tile
    import concourse.mybir as mybir

    f32 = mybir.dt.float32
    AF = mybir.ActivationFunctionType
    OP = mybir.AluOpType
    AX = mybir.AxisListType

    CW = 3 * HW + 128 + 5 * CL   # packed const width
    nc = bacc.Bacc("TRN2", target_bir_lowering=False)
    cst_d = nc.dram_tensor("cst", [128, CW], f32, kind="ExternalInput")
    out_d = nc.dram_tensor("out", [2 * CL, HW], f32, kind="ExternalOutput")

    with ExitStack() as ctx:
        tc = ctx.enter_context(tile.TileContext(nc))
        cpool = ctx.enter_context(tc.tile_pool(name="const", bufs=1))
        work = ctx.enter_context(tc.tile_pool(name="work", bufs=2))
        psum = ctx.enter_context(tc.tile_pool(name="psum", bufs=8, space="PSUM"))
        outp = ctx.enter_context(tc.tile_pool(name="outp", bufs=2))

        CST = cpool.tile([128, CW], f32, tag="CST")
        nc.gpsimd.dma_start(CST[:, :], cst_d[:, :])
        V = CST[:, 0:HW]
        FNT = CST[:, HW : 2 * HW]
        NM = CST[:, 2 * HW : 3 * HW]
        ID = CST[:, 3 * HW : 3 * HW + 128]
        SCAL = CST[:, 3 * HW + 128 :]
        A2 = SCAL[:, 0 * CL : 1 * CL]
        S2 = SCAL[:, 1 * CL : 2 * CL]
        SR = SCAL[:, 2 * CL : 3 * CL]
        BR = SCAL[:, 3 * CL : 4 * CL]
        LX = SCAL[:, 4 * CL : 5 * CL]

        nchunk = HW // FREE_CHUNK
        blk_per_chunk = FREE_CHUNK // 128          # 16
        grp_per_chunk = blk_per_chunk // 4         # 4 (one PSUM bank each)

        for it in range(CL):
            a2 = A2[:, it : it + 1]
            s2 = S2[:, it : it + 1]
            sr = SR[:, it : it + 1]
            br = BR[:, it : it + 1]
            lx = LX[:, it : it + 1]
            o_t = outp.tile([128, NBLK, 2], f32, tag="o_t")
            for ch in range(nchunk):
                sl = slice(ch * FREE_CHUNK, (ch + 1) * FREE_CHUNK)
                # u = theta/2pi - a/2pi
                u = work.tile([128, FREE_CHUNK], f32, tag="u")
                nc.scalar.activation(u[:], V[:, sl], AF.Identity, bias=a2)
                # rr = round(u)  via (u + C) - C
                rr = work.tile([128, FREE_CHUNK], f32, tag="rr")
                nc.gpsimd.tensor_scalar(rr[:], u[:], CBIG, CBIG, OP.add, OP.subtract)
                # wu = u - round(u)  in [-0.5, 0.5]
                wu = work.tile([128, FREE_CHUNK], f32, tag="wu")
                nc.vector.tensor_tensor(wu[:], u[:], rr[:], OP.subtract)
                # mask: zero the angle at each origin's own pixel
                wm = work.tile([128, FREE_CHUNK], f32, tag="wm")
                nc.gpsimd.tensor_tensor(wm[:], wu[:], NM[:, sl], OP.mult)
                # sqa = (2pi*sqrt(inv2av) * wm)^2
                sqa = work.tile([128, FREE_CHUNK], f32, tag="sqa")
                nc.scalar.activation(sqa[:], wm[:], AF.Square, scale=s2)
                # rdn = (sqrt(inv2rv)*fn - rm*sqrt(inv2rv))^2
                rdn = work.tile([128, FREE_CHUNK], f32, tag="rdn")
                nc.scalar.activation(rdn[:], FNT[:, sl], AF.Square, scale=sr, bias=br)
                # t = sqa + rdn ; s = -t + ln x
                tt = work.tile([128, FREE_CHUNK], f32, tag="tt")
                nc.vector.tensor_tensor(tt[:], sqa[:], rdn[:], OP.add)
                s = work.tile([128, FREE_CHUNK], f32, tag="s")
                nc.scalar.activation(s[:], tt[:], AF.Identity, scale=-1.0, bias=lx)
                # transpose 128x128 blocks to PSUM, max-reduce origins
                for g in range(grp_per_chunk):
                    ps = psum.tile([128, 512], f32, tag="ps")
                    for l in range(4):
                        nc.tensor.transpose(
                            ps[:, l * 128 : (l + 1) * 128],
                            s[:, (g * 4 + l) * 128 : (g * 4 + l + 1) * 128],
                            ID[:, :],
                        )
                    red_in = ps[:, :].rearrange("p (l r o) -> p l r o", l=4, r=2, o=64)
                    b0 = ch * blk_per_chunk + g * 4
                    nc.vector.tensor_reduce(
                        o_t[:, b0 : b0 + 4, :], red_in, axis=AX.X, op=OP.max
                    )
            o_e = outp.tile([128, NBLK, 2], f32, tag="o_e")
            nc.scalar.activation(o_e[:, :, :], o_t[:, :, :], AF.Exp)
            for pair in range(2):
                row = pair * CL + it
                nc.sync.dma_start(
                    out_d[row].rearrange("(blk p) -> p blk", p=128),
                    o_e[:, :, pair],
                )
    nc.finalize()
    return nc


def _host_scalars(x, radius_mean, angle_mean, radius_std, angle_std):
    """Per-core scalar tables [128, CL], fp64->fp32. partition = b*64 + o."""
    inv2rv = 1.0 / (2.0 * (radius_std.astype(np.float64) ** 2 + 0.01))   # [C]
    inv2av = 1.0 / (2.0 * (angle_std.astype(np.float64) ** 2 + 0.0001))  # [C]
    rm = np.abs(radius_mean.astype(np.float64)).reshape(B, C, NO)
    am = angle_mean.astype(np.float64).reshape(B, C, NO)
    xx = np.maximum(x.astype(np.float64).reshape(B, C, NO), 1e-30)
    per_core = []
    for k in range(NCORES):
        cs = np.arange(k * CL, (k + 1) * CL)
        a2 = np.zeros((128, CL)); s2 = np.zeros((128, CL))
        sr = np.zeros((128, CL)); br = np.zeros((128, CL))
        lxv = np.zeros((128, CL))
        for itc, c in enumerate(cs):
            srt = np.sqrt(inv2rv[c])
            for b in range(B):
                p = slice(b * NO, (b + 1) * NO)
                a2[p, itc] = -am[b, c] / TWO_PI
                s2[p, itc] = TWO_PI * np.sqrt(inv2av[c])
                sr[p, itc] = srt
                br[p, itc] = -rm[b, c] * srt
                lxv[p, itc] = np.log(xx[b, c])
        f = lambda a: np.ascontiguousarray(a.astype(np.float32))
        per_core.append(dict(a2=f(a2), s2=f(s2), sr=f(sr), br=f(br), lx=f(lxv)))
    return per_core


def kernel(x, radius_mean, angle_mean, radius_std, angle_std, _trace=False,
           _tmpdir=None):
    from concourse.bass_utils import run_bass_kernel_spmd

    if "nc" not in _built:
        _built["nc"] = _build_bass()
        _built["fields"] = _host_fields()
    nc = _built["nc"]
    v, fn, nm = _built["fields"]
    fld = np.concatenate([v, fn, nm], axis=1)          # [64, 3*HW]
    fld2 = np.concatenate([fld, fld], axis=0)          # [128, 3*HW]
    ident = np.eye(128, dtype=np.float32)
    sc = _host_scalars(x, radius_mean, angle_mean, radius_std, angle_std)
    in_maps = []
    for k in range(NCORES):
        s = sc[k]
        scal = np.concatenate(
            [s["a2"], s["s2"], s["sr"], s["br"], s["lx"]], axis=1)
        cst = np.ascontiguousarray(
            np.concatenate([fld2, ident, scal], axis=1))
        in_maps.append({"cst": cst})
    res = run_bass_kernel_spmd(nc, in_maps, core_ids=list(range(NCORES)),
                               trace=_trace, tmpdir=_tmpdir)
    if _trace:
        return res
    out = np.empty((B, C, H, W), dtype=np.float32)
    for k in range(NCORES):
        r = res.results[k]["out"].reshape(B, CL, H, W)
        out[:, k * CL : (k + 1) * CL] = r
    return out



# revision 6
# speedup vs baseline: 2.7347x; 2.7347x over previous
"""LongRangeProj Bass kernel for TRN2 (8 NeuronCores, channel-sharded).

Math: out[b,c,h,w] = max_{o=(i,j)} x[b,c,o] * exp(-(inv2rv*(fn-|rm|)^2
                                                   + inv2av*wrap(theta-a)^2))
with fn/theta = polar coords of pixel (h,w) around origin o, and the angle
term forced to 1 at the origin pixel itself.  exp is monotone, so the
reduction happens on the exponent: sl = sqa + rdn - ln x, reduced with MIN
over origins, then out = exp(-min).

The origin-pixel mask is applied AFTER the reduce: the correct value at
origin o's own pixel is rdn(0) - ln x = br^2 - lx, precomputed host-side in
a per-channel FIX table (+1e30 elsewhere), folded in with one tiny min op.

Angle wrap uses the fp32 magic-constant round trick entirely on DVE
tensor_scalar ops (2x mode, ~1.15us per [128x2048]):
    t1  = (v + a2) + CBIG          # rounds to integer near CBIG
    rra = (t1 - CBIG) - a2         # round(u) - a2
    wu  = v - rra                  # u - round(u) in [-0.5, 0.5]  (GPSIMD)
    sqa = (s2*wu)^2                # ACT Square
    rdn = (sr*fn + br)^2           # ACT Square
    sl  = (sqa - lx) + rdn         # DVE scalar_tensor_tensor
Engines per [128x2048] chunk: DVE 2 ts + 1 stt + 4 reduce (~7us),
ACT 2 Square (~4us), GPSIMD 1 tensor_tensor (~4.5us), PE 32 transposes
(~6.2us).  Channel loop software-pipelined so the DVE combine of chunk k
runs while GPSIMD/ACT produce chunk k+1.
"""

import numpy as np
from contextlib import ExitStack

B, C, NH, NW, H, W = 2, 64, 8, 8, 64, 64
STRIDE = 8
NCORES = 8
CL = C // NCORES          # channels per core
HW = H * W                # 4096
NO = NH * NW              # 64 origins
FREE_CHUNK = 2048
NBLK = HW // 128          # 32 pixel blocks of 128
CBIG = float(1.5 * 2 ** 23)   # fp32 round-to-nearest magic constant
TWO_PI = 2.0 * np.pi
NCHUNK = HW // FREE_CHUNK     # 2
BPC = FREE_CHUNK // 128       # 16 blocks per chunk
GPC = BPC // 4                # 4 psum groups per chunk

_built = {}


def _host_fields():
    """Constant geometric fields in [NO, HW] layout, fp32."""
    oy = np.arange(NH, dtype=np.float64) * STRIDE
    ox = np.arange(NW, dtype=np.float64) * STRIDE
    yg = np.arange(H, dtype=np.float64)
    xg = np.arange(W, dtype=np.float64)
    fy = yg[None, :] - oy[:, None]                      # [NH, H]
    fx = xg[None, :] - ox[:, None]                      # [NW, W]
    FY = np.broadcast_to(fy[:, None, :, None], (NH, NW, H, W))
    FX = np.broadcast_to(fx[None, :, None, :], (NH, NW, H, W))
    fn = np.sqrt(FX * FX + FY * FY)
    theta = np.arctan2(FY, FX)
    v = theta / TWO_PI
    rs = lambda a: np.ascontiguousarray(a.reshape(NO, HW).astype(np.float32))
    return rs(v), rs(fn)


def _build_bass():
    import concourse.bass as bass
    import concourse.bacc as bacc
    import concourse.tile as tile
    import concourse.mybir as mybir

    f32 = mybir.dt.float32
    AF = mybir.ActivationFunctionType
    OP = mybir.AluOpType
    AX = mybir.AxisListType

    CW = 2 * HW + 128 + 5 * CL + 64 * CL   # packed const width
    nc = bacc.Bacc("TRN2", target_bir_lowering=False)
    cst_d = nc.dram_tensor("cst", [128, CW], f32, kind="ExternalInput")
    out_d = nc.dram_tensor("out", [2 * CL, HW], f32, kind="ExternalOutput")

    with ExitStack() as ctx:
        tc = ctx.enter_context(tile.TileContext(nc))
        cpool = ctx.enter_context(tc.tile_pool(name="const", bufs=1))
        front = ctx.enter_context(tc.tile_pool(name="front", bufs=2))
        back = ctx.enter_context(tc.tile_pool(name="back", bufs=2))
        slp = ctx.enter_context(tc.tile_pool(name="slp", bufs=2))
        psum = ctx.enter_context(tc.tile_pool(name="psum", bufs=8, space="PSUM"))
        outp = ctx.enter_context(tc.tile_pool(name="outp", bufs=2))

        CST = cpool.tile([128, CW], f32, tag="CST")
        nc.gpsimd.dma_start(CST[:, :], cst_d[:, :])
        V = CST[:, 0:HW]
        FNT = CST[:, HW : 2 * HW]
        ID = CST[:, 2 * HW : 2 * HW + 128]
        SCAL = CST[:, 2 * HW + 128 :]
        A2 = SCAL[:, 0 * CL : 1 * CL]
        S2 = SCAL[:, 1 * CL : 2 * CL]
        SR = SCAL[:, 2 * CL : 3 * CL]
        BR = SCAL[:, 3 * CL : 4 * CL]
        LX = SCAL[:, 4 * CL : 5 * CL]
        FIX = SCAL[:, 5 * CL :]

        # software pipeline: stage A (chunk production) for iteration k runs
        # alongside stage B (combine+transpose+reduce) for iteration k-1.
        steps = []   # (it, ch) flat iteration order
        for it in range(CL):
            for ch in range(NCHUNK):
                steps.append((it, ch))

        o_ts = {}    # per-channel output accumulators
        pend = None  # (it, ch, sqa, rdn)

        def stage_a(it, ch):
            a2 = A2[:, it : it + 1]
            s2 = S2[:, it : it + 1]
            sr = SR[:, it : it + 1]
            br = BR[:, it : it + 1]
            sl_ = slice(ch * FREE_CHUNK, (ch + 1) * FREE_CHUNK)
            # DVE: t1 = (v + a2) + CBIG ; rra = (t1 - CBIG) - a2
            t1 = front.tile([128, FREE_CHUNK], f32, tag="t1")
            nc.vector.tensor_scalar(t1[:], V[:, sl_], a2, CBIG, OP.add, OP.add)
            rra = front.tile([128, FREE_CHUNK], f32, tag="rra")
            nc.vector.tensor_scalar(rra[:], t1[:], CBIG, a2,
                                    OP.subtract, OP.subtract)
            # ACT: rdn first (no deps beyond consts)
            rdn = back.tile([128, FREE_CHUNK], f32, tag="rdn")
            nc.scalar.activation(rdn[:], FNT[:, sl_], AF.Square,
                                 scale=sr, bias=br)
            # GPSIMD: wu = v - rra
            wu = front.tile([128, FREE_CHUNK], f32, tag="wu")
            nc.gpsimd.tensor_tensor(wu[:], V[:, sl_], rra[:], OP.subtract)
            # ACT: sqa = (s2*wu)^2
            sqa = back.tile([128, FREE_CHUNK], f32, tag="sqa")
            nc.scalar.activation(sqa[:], wu[:], AF.Square, scale=s2)
            return sqa, rdn

        def stage_b(it, ch, sqa, rdn):
            lx = LX[:, it : it + 1]
            sl = slp.tile([128, FREE_CHUNK], f32, tag="sl")
            nc.vector.scalar_tensor_tensor(sl[:], sqa[:], lx, rdn[:],
                                           OP.subtract, OP.add)
            o_t = o_ts[it]
            for g in range(GPC):
                ps = psum.tile([128, 512], f32, tag="ps")
                for l in range(4):
                    nc.tensor.transpose(
                        ps[:, l * 128 : (l + 1) * 128],
                        sl[:, (g * 4 + l) * 128 : (g * 4 + l + 1) * 128],
                        ID[:, :],
                    )
                red_in = ps[:, :].rearrange("p (l r o) -> p l r o",
                                            l=4, r=2, o=64)
                b0 = ch * BPC + g * 4
                nc.vector.tensor_reduce(
                    o_t[:, b0 : b0 + 4, :], red_in, axis=AX.X, op=OP.min
                )

        def finish_channel(it):
            fix = FIX[:, it * 64 : (it + 1) * 64].rearrange(
                "p (blk r) -> p blk r", blk=NBLK, r=2)
            o_t = o_ts.pop(it)
            o_f = outp.tile([128, NBLK, 2], f32, tag="o_f")
            nc.vector.scalar_tensor_tensor(o_f[:, :, :], o_t[:, :, :], 0.0,
                                           fix, OP.add, OP.min)
            o_e = outp.tile([128, NBLK, 2], f32, tag="o_e")
            nc.scalar.activation(o_e[:, :, :], o_f[:, :, :], AF.Exp,
                                 scale=-1.0)
            for pair in range(2):
                row = pair * CL + it
                nc.sync.dma_start(
                    out_d[row].rearrange("(blk p) -> p blk", p=128),
                    o_e[:, :, pair],
                )

        for k, (it, ch) in enumerate(steps):
            if ch == 0:
                o_t = outp.tile([128, NBLK, 2], f32, tag="o_t")
                o_ts[it] = o_t
            sqa, rdn = stage_a(it, ch)
            if pend is not None:
                stage_b(*pend)
                if pend[1] == NCHUNK - 1:
                    finish_channel(pend[0])
            pend = (it, ch, sqa, rdn)
        stage_b(*pend)
        finish_channel(pend[0])
    nc.finalize()
    return nc


def _host_scalars(x, radius_mean, angle_mean, radius_std, angle_std):
    """Per-core scalar tables [128, CL] + FIX [128, 64*CL].

    partition = b*64 + o.  FIX[8j, 4i*2 + b, for channel slot] = br^2 - lx
    at origin o=(i,j)'s own pixel (h=8i, w=8j -> block 4i, partition 8j).
    """
    inv2rv = 1.0 / (2.0 * (radius_std.astype(np.float64) ** 2 + 0.01))   # [C]
    inv2av = 1.0 / (2.0 * (angle_std.astype(np.float64) ** 2 + 0.0001))  # [C]
    rm = np.abs(radius_mean.astype(np.float64)).reshape(B, C, NO)
    am = angle_mean.astype(np.float64).reshape(B, C, NO)
    xx = np.maximum(x.astype(np.float64).reshape(B, C, NO), 1e-30)
    per_core = []
    for k in range(NCORES):
        cs = np.arange(k * CL, (k + 1) * CL)
        a2 = np.zeros((128, CL)); s2 = np.zeros((128, CL))
        sr = np.zeros((128, CL)); br = np.zeros((128, CL))
        lxv = np.zeros((128, CL))
        fix = np.full((128, CL, NBLK, 2), 1e30)
        for itc, c in enumerate(cs):
            srt = np.sqrt(inv2rv[c])
            for b in range(B):
                p = slice(b * NO, (b + 1) * NO)
                a2[p, itc] = -am[b, c] / TWO_PI
                s2[p, itc] = TWO_PI * np.sqrt(inv2av[c])
                sr[p, itc] = srt
                br[p, itc] = -rm[b, c] * srt
                lxv[p, itc] = np.log(xx[b, c])
                for i in range(NH):
                    for j in range(NW):
                        o = i * NW + j
                        brv = np.float32(-rm[b, c, o] * srt)
                        lv = np.float32(np.log(xx[b, c, o]))
                        fix[8 * j, itc, 4 * i, b] = (
                            np.float32(brv * brv) - lv)
        f = lambda a: np.ascontiguousarray(a.astype(np.float32))
        per_core.append(dict(a2=f(a2), s2=f(s2), sr=f(sr), br=f(br),
                             lx=f(lxv),
                             fix=f(fix.reshape(128, CL * NBLK * 2))))
    return per_core


def kernel(x, radius_mean, angle_mean, radius_std, angle_std, _trace=False,
           _tmpdir=None):
    from concourse.bass_utils import run_bass_kernel_spmd

    if "nc" not in _built:
        _built["nc"] = _build_bass()
        _built["fields"] = _host_fields()
    nc = _built["nc"]
    v, fn = _built["fields"]
    fld = np.concatenate([v, fn], axis=1)              # [64, 2*HW]
    fld2 = np.concatenate([fld, fld], axis=0)          # [128, 2*HW]
    ident = np.eye(128, dtype=np.float32)
    sc = _host_scalars(x, radius_mean, angle_mean, radius_std, angle_std)
    in_maps = []
    for k in range(NCORES):
        s = sc[k]
        scal = np.concatenate(
            [s["a2"], s["s2"], s["sr"], s["br"], s["lx"], s["fix"]], axis=1)
        cst = np.ascontiguousarray(
            np.concatenate([fld2, ident, scal], axis=1))
        in_maps.append({"cst": cst})
    res = run_bass_kernel_spmd(nc, in_maps, core_ids=list(range(NCORES)),
                               trace=_trace, tmpdir=_tmpdir)
    if _trace:
        return res
    out = np.empty((B, C, H, W), dtype=np.float32)
    for k in range(NCORES):
        r = res.results[k]["out"].reshape(B, CL, H, W)
        out[:, k * CL : (k + 1) * CL] = r
    return out


# revision 9
# speedup vs baseline: 3.3433x; 1.2226x over previous
"""LongRangeProj Bass kernel for TRN2 (8 NeuronCores, channel-sharded).

Math: out[b,c,h,w] = max_{o=(i,j)} x[b,c,o] * exp(-(inv2rv*(fn-|rm|)^2
                                                   + inv2av*wrap(theta-a)^2))
with fn/theta = polar coords of pixel (h,w) around origin o, and the angle
term forced to 1 at the origin pixel itself.  exp is monotone, so the
reduction happens on the exponent: sl = sqa + rdn - ln x, reduced with MIN
over origins, then out = exp(-min).

The origin-pixel mask is applied AFTER the reduce: the correct value at
origin o's own pixel is rdn(0) - ln x = br^2 - lx, precomputed host-side in
a per-channel FIX table (+1e30 elsewhere), folded in with one tiny min op.

Angle wrap uses the fp32 magic-constant round trick entirely on DVE
tensor_scalar ops (2x mode, ~1.15us per [128x2048]):
    t1  = (v + a2) + CBIG          # rounds to integer near CBIG
    rra = (t1 - CBIG) - a2         # round(u) - a2
    wu  = v - rra                  # u - round(u) in [-0.5, 0.5]  (GPSIMD)
    sqa = (s2*wu)^2                # ACT Square
    rdn = (sr*fn + br)^2           # ACT Square
    sl  = (sqa - lx) + rdn         # DVE scalar_tensor_tensor
Engines per [128x2048] chunk: DVE 2 ts + 1 stt + 4 reduce (~7us),
ACT 2 Square (~4us), GPSIMD 1 tensor_tensor (~4.5us), PE 32 transposes
(~6.2us).  Channel loop software-pipelined so the DVE combine of chunk k
runs while GPSIMD/ACT produce chunk k+1.
"""

import numpy as np
from contextlib import ExitStack

B, C, NH, NW, H, W = 2, 64, 8, 8, 64, 64
STRIDE = 8
NCORES = 8
CL = C // NCORES          # channels per core
HW = H * W                # 4096
NO = NH * NW              # 64 origins
FREE_CHUNK = 2048
NBLK = HW // 128          # 32 pixel blocks of 128
CBIG = float(1.5 * 2 ** 23)   # fp32 round-to-nearest magic constant
TWO_PI = 2.0 * np.pi
NCHUNK = HW // FREE_CHUNK     # 2
BPC = FREE_CHUNK // 128       # 16 blocks per chunk
GPC = BPC // 4                # 4 psum groups per chunk

_built = {}


def _host_fields():
    """Constant geometric fields in [NO, HW] layout, fp32."""
    oy = np.arange(NH, dtype=np.float64) * STRIDE
    ox = np.arange(NW, dtype=np.float64) * STRIDE
    yg = np.arange(H, dtype=np.float64)
    xg = np.arange(W, dtype=np.float64)
    fy = yg[None, :] - oy[:, None]                      # [NH, H]
    fx = xg[None, :] - ox[:, None]                      # [NW, W]
    FY = np.broadcast_to(fy[:, None, :, None], (NH, NW, H, W))
    FX = np.broadcast_to(fx[None, :, None, :], (NH, NW, H, W))
    fn = np.sqrt(FX * FX + FY * FY)
    theta = np.arctan2(FY, FX)
    v = theta / TWO_PI
    rs = lambda a: np.ascontiguousarray(a.reshape(NO, HW).astype(np.float32))
    return rs(v), rs(fn)


def _build_bass():
    import concourse.bass as bass
    import concourse.bacc as bacc
    import concourse.tile as tile
    import concourse.mybir as mybir

    f32 = mybir.dt.float32
    AF = mybir.ActivationFunctionType
    OP = mybir.AluOpType
    AX = mybir.AxisListType

    CW = 2 * HW + 128 + 5 * CL + 64 * CL   # packed const width
    nc = bacc.Bacc("TRN2", target_bir_lowering=False)
    cst_d = nc.dram_tensor("cst", [128, CW], f32, kind="ExternalInput")
    # SBUF-natural layout: [channel, partition(pixel-in-block), blk, batch];
    # host reorders to [B, CL, H, W].  Contiguous 256B per partition per DMA.
    out_d = nc.dram_tensor("out", [CL, 128, NBLK, 2], f32,
                           kind="ExternalOutput")

    with ExitStack() as ctx:
        tc = ctx.enter_context(tile.TileContext(nc))
        cpool = ctx.enter_context(tc.tile_pool(name="const", bufs=1))
        front = ctx.enter_context(tc.tile_pool(name="front", bufs=2))
        back = ctx.enter_context(tc.tile_pool(name="back", bufs=2))
        slp = ctx.enter_context(tc.tile_pool(name="slp", bufs=2))
        psum = ctx.enter_context(tc.tile_pool(name="psum", bufs=8, space="PSUM"))
        outp = ctx.enter_context(tc.tile_pool(name="outp", bufs=2))

        CST = cpool.tile([128, CW], f32, tag="CST")
        nc.gpsimd.dma_start(CST[:, :], cst_d[:, :])
        V = CST[:, 0:HW]
        FNT = CST[:, HW : 2 * HW]
        ID = CST[:, 2 * HW : 2 * HW + 128]
        SCAL = CST[:, 2 * HW + 128 :]
        A2 = SCAL[:, 0 * CL : 1 * CL]
        S2 = SCAL[:, 1 * CL : 2 * CL]
        SR = SCAL[:, 2 * CL : 3 * CL]
        BR = SCAL[:, 3 * CL : 4 * CL]
        LX = SCAL[:, 4 * CL : 5 * CL]
        FIX = SCAL[:, 5 * CL :]

        # software pipeline: stage A (chunk production) for iteration k runs
        # alongside stage B (combine+transpose+reduce) for iteration k-1.
        steps = []   # (it, ch) flat iteration order
        for it in range(CL):
            for ch in range(NCHUNK):
                steps.append((it, ch))

        o_ts = {}    # per-channel output accumulators
        pend = None  # (it, ch, sqa, rdn)

        def stage_a(it, ch):
            a2 = A2[:, it : it + 1]
            s2 = S2[:, it : it + 1]
            sr = SR[:, it : it + 1]
            br = BR[:, it : it + 1]
            sl_ = slice(ch * FREE_CHUNK, (ch + 1) * FREE_CHUNK)
            # DVE: t1 = (v + a2) + CBIG ; rra = (t1 - CBIG) - a2
            t1 = front.tile([128, FREE_CHUNK], f32, tag="t1")
            nc.vector.tensor_scalar(t1[:], V[:, sl_], a2, CBIG, OP.add, OP.add)
            rra = front.tile([128, FREE_CHUNK], f32, tag="rra")
            nc.vector.tensor_scalar(rra[:], t1[:], CBIG, a2,
                                    OP.subtract, OP.subtract)
            # ACT: rdn first (no deps beyond consts)
            rdn = back.tile([128, FREE_CHUNK], f32, tag="rdn")
            nc.scalar.activation(rdn[:], FNT[:, sl_], AF.Square,
                                 scale=sr, bias=br)
            # GPSIMD: wu = v - rra
            wu = front.tile([128, FREE_CHUNK], f32, tag="wu")
            nc.gpsimd.tensor_tensor(wu[:], V[:, sl_], rra[:], OP.subtract)
            # ACT: sqa = (s2*wu)^2
            sqa = back.tile([128, FREE_CHUNK], f32, tag="sqa")
            nc.scalar.activation(sqa[:], wu[:], AF.Square, scale=s2)
            return sqa, rdn

        def stage_b(it, ch, sqa, rdn):
            lx = LX[:, it : it + 1]
            sl = slp.tile([128, FREE_CHUNK], f32, tag="sl")
            nc.vector.scalar_tensor_tensor(sl[:], sqa[:], lx, rdn[:],
                                           OP.subtract, OP.add)
            o_t = o_ts[it]
            for g in range(GPC):
                ps = psum.tile([128, 512], f32, tag="ps")
                for l in range(4):
                    nc.tensor.transpose(
                        ps[:, l * 128 : (l + 1) * 128],
                        sl[:, (g * 4 + l) * 128 : (g * 4 + l + 1) * 128],
                        ID[:, :],
                    )
                red_in = ps[:, :].rearrange("p (l r o) -> p l r o",
                                            l=4, r=2, o=64)
                b0 = ch * BPC + g * 4
                nc.vector.tensor_reduce(
                    o_t[:, b0 : b0 + 4, :], red_in, axis=AX.X, op=OP.min
                )

        def finish_channel(it):
            fix = FIX[:, it * 64 : (it + 1) * 64].rearrange(
                "p (blk r) -> p blk r", blk=NBLK, r=2)
            o_t = o_ts.pop(it)
            o_f = outp.tile([128, NBLK, 2], f32, tag="o_f")
            nc.vector.scalar_tensor_tensor(o_f[:, :, :], o_t[:, :, :], 0.0,
                                           fix, OP.add, OP.min)
            o_e = outp.tile([128, NBLK, 2], f32, tag="o_e")
            nc.scalar.activation(o_e[:, :, :], o_f[:, :, :], AF.Exp,
                                 scale=-1.0)
            nc.sync.dma_start(out_d[it], o_e[:, :, :])

        for k, (it, ch) in enumerate(steps):
            if ch == 0:
                o_t = outp.tile([128, NBLK, 2], f32, tag="o_t")
                o_ts[it] = o_t
            sqa, rdn = stage_a(it, ch)
            if pend is not None:
                stage_b(*pend)
                if pend[1] == NCHUNK - 1:
                    finish_channel(pend[0])
            pend = (it, ch, sqa, rdn)
        stage_b(*pend)
        finish_channel(pend[0])
    nc.finalize()
    return nc


def _host_scalars(x, radius_mean, angle_mean, radius_std, angle_std):
    """Per-core scalar tables [128, CL] + FIX [128, 64*CL].

    partition = b*64 + o.  FIX[8j, 4i*2 + b, for channel slot] = br^2 - lx
    at origin o=(i,j)'s own pixel (h=8i, w=8j -> block 4i, partition 8j).
    """
    inv2rv = 1.0 / (2.0 * (radius_std.astype(np.float64) ** 2 + 0.01))   # [C]
    inv2av = 1.0 / (2.0 * (angle_std.astype(np.float64) ** 2 + 0.0001))  # [C]
    rm = np.abs(radius_mean.astype(np.float64)).reshape(B, C, NO)
    am = angle_mean.astype(np.float64).reshape(B, C, NO)
    xx = np.maximum(x.astype(np.float64).reshape(B, C, NO), 1e-30)
    per_core = []
    for k in range(NCORES):
        cs = np.arange(k * CL, (k + 1) * CL)
        a2 = np.zeros((128, CL)); s2 = np.zeros((128, CL))
        sr = np.zeros((128, CL)); br = np.zeros((128, CL))
        lxv = np.zeros((128, CL))
        fix = np.full((128, CL, NBLK, 2), 1e30)
        for itc, c in enumerate(cs):
            srt = np.sqrt(inv2rv[c])
            for b in range(B):
                p = slice(b * NO, (b + 1) * NO)
                a2[p, itc] = -am[b, c] / TWO_PI
                s2[p, itc] = TWO_PI * np.sqrt(inv2av[c])
                sr[p, itc] = srt
                br[p, itc] = -rm[b, c] * srt
                lxv[p, itc] = np.log(xx[b, c])
                for i in range(NH):
                    for j in range(NW):
                        o = i * NW + j
                        brv = np.float32(-rm[b, c, o] * srt)
                        lv = np.float32(np.log(xx[b, c, o]))
                        fix[8 * j, itc, 4 * i, b] = (
                            np.float32(brv * brv) - lv)
        f = lambda a: np.ascontiguousarray(a.astype(np.float32))
        per_core.append(dict(a2=f(a2), s2=f(s2), sr=f(sr), br=f(br),
                             lx=f(lxv),
                             fix=f(fix.reshape(128, CL * NBLK * 2))))
    return per_core


def kernel(x, radius_mean, angle_mean, radius_std, angle_std, _trace=False,
           _tmpdir=None):
    from concourse.bass_utils import run_bass_kernel_spmd

    if "nc" not in _built:
        _built["nc"] = _build_bass()
        _built["fields"] = _host_fields()
    nc = _built["nc"]
    v, fn = _built["fields"]
    fld = np.concatenate([v, fn], axis=1)              # [64, 2*HW]
    fld2 = np.concatenate([fld, fld], axis=0)          # [128, 2*HW]
    ident = np.eye(128, dtype=np.float32)
    sc = _host_scalars(x, radius_mean, angle_mean, radius_std, angle_std)
    in_maps = []
    for k in range(NCORES):
        s = sc[k]
        scal = np.concatenate(
            [s["a2"], s["s2"], s["sr"], s["br"], s["lx"], s["fix"]], axis=1)
        cst = np.ascontiguousarray(
            np.concatenate([fld2, ident, scal], axis=1))
        in_maps.append({"cst": cst})
    res = run_bass_kernel_spmd(nc, in_maps, core_ids=list(range(NCORES)),
                               trace=_trace, tmpdir=_tmpdir)
    if _trace:
        return res
    out = np.empty((B, C, H, W), dtype=np.float32)
    for k in range(NCORES):
        r = res.results[k]["out"]          # [CL, 128, NBLK, 2]
        # value at [it, p, blk, b] is pixel blk*128+p of batch b, channel it
        r = r.transpose(3, 0, 2, 1).reshape(B, CL, H, W)
        out[:, k * CL : (k + 1) * CL] = r
    return out
